# revision 47
# baseline (speedup 1.0000x reference)
# Adaptive softmax (3-cluster) on 8 TRN2 NeuronCores.
#
# Strategy (moe_routing): each token only needs its own cluster's pipeline.
# Host-side we sort tokens by cluster (pure data movement), shard each
# cluster's token segment evenly over the 8 cores, and pad each per-core
# segment to a static capacity so the Bass graph stays shape-static.
# Per core, per cluster c with nt tokens / pd proj dim / C classes:
#   hiddenT[pd, nt] = p_c @ x_shard^T            (PE, bf16)
#   hidden  [nt, pd]                             (PE, bf16; token-major)
#   logits  [nt, C] = hidden @ w_c^T             (PE, fp8 DoubleRow, chunked)
#   sumexp  [nt]    = sum_C exp(logits)          (ScalarE exp -> DVE row-sum)
#   logit_t [nt]    = rowdot(hidden, w_c[tgt]) + b_c[tgt]    (DVE, bf16)
#   nll     [nt]    = ln(sumexp) - logit_t
# The big logits GEMM runs in fp8e4m3 DoubleRow (2 rows/PE cell = 2x MACs).
# fp8 range handling: weights are pre-scaled x64 and hidden x4 (powers of
# two, lossless), and the exp activation rescales with scale=1/256.
# No max-subtraction is needed: |logits| <~ 4 for this problem's scales.
# Classes are padded to uniform 512-wide chunks with zero weights; each
# padded class contributes exactly exp(0)=1, corrected via the Ln pre-bias.
# The target-row weights w_c[tgt] are gathered on host (data movement only).
# No collectives: pure data parallelism; host gathers/unpermutes/sums.

import numpy as np
import ml_dtypes
from contextlib import ExitStack

import concourse.bass as bass
import concourse.bacc as bacc
import concourse.mybir as mybir
import concourse.tile as tile
from concourse.bass_utils import run_bass_kernel_spmd

BF16 = mybir.dt.bfloat16
FP8 = mybir.dt.float8e4
F32 = mybir.dt.float32
bf16 = ml_dtypes.bfloat16
fp8 = ml_dtypes.float8_e4m3

VOCAB = 50257
D = 1024           # input dim
KD = D // 128      # k-tiles over input dim
CUT = [0, 10000, 30000, VOCAB]
PD = [1024, 512, 256]            # per-cluster projection dims
KP = [p // 128 for p in PD]      # k-tiles over proj dim
CSIZE = [CUT[i + 1] - CUT[i] for i in range(3)]
NCORES = 8
CAP = [256, 512, 512]            # per-core token capacity per cluster (padded)
TILES = [c // 128 for c in CAP]
SLOT0 = [0, 256, 768]            # slot offset of each cluster's segment
TOT = sum(CAP)                   # 1280 padded tokens per core
NT = TOT // 128                  # 10 token tiles per core
CHUNK = 512                      # class chunk (one PSUM bank of f32)

HSCALE = 4.0                     # hidden fp8 pre-scale (power of 2)
WSCALE = 64.0                    # weight fp8 pre-scale (power of 2)
EXP_SCALE = 1.0 / (HSCALE * WSCALE)

# class padding to uniform chunks
CPAD = [-(-C // CHUNK) * CHUNK for C in CSIZE]
NPAD = [CPAD[i] - CSIZE[i] for i in range(3)]
CHUNKS = [[(off, CHUNK) for off in range(0, CPAD[i], CHUNK)] for i in range(3)]

# cluster processing order: smallest input first (shrinks the pre-matmul
# head), c0 last (smallest tail)
ORDER = [2, 1, 0]

_GRAPH_CACHE = {}


def _build_graph():
    # Bacc (not plain Bass): its compile() pass splits semaphore waits into
    # event-semaphore carriers, satisfying TRN2's 1-wait-per-instruction limit.
    nc = bacc.Bacc(trn_type="TRN2", target_bir_lowering=False)

    xT_d = nc.dram_tensor("xT", [128, KD, TOT], BF16, kind="ExternalInput")
    pT_d = [
        nc.dram_tensor(f"pT{i}", [128, KD, PD[i]], BF16, kind="ExternalInput")
        for i in range(3)
    ]
    F = [KP[i] * CPAD[i] for i in range(3)]
    wT_d = [
        nc.dram_tensor(f"wT{i}", [128, F[i]], FP8, kind="ExternalInput")
        for i in range(3)
    ]
    # wg carries the gathered target-row weights plus one bias column
    wg_d = [
        nc.dram_tensor(f"wg{i}", [TILES[i], 128, PD[i] + 1], BF16, kind="ExternalInput")
        for i in range(3)
    ]
    out_d = nc.dram_tensor("out", [NT, 128], F32, kind="ExternalOutput")

    Exp = mybir.ActivationFunctionType.Exp
    Ln = mybir.ActivationFunctionType.Ln
    X = mybir.AxisListType.X
    DR = mybir.MatmulPerfMode.DoubleRow

    with ExitStack() as ctx:
        tc = ctx.enter_context(tile.TileContext(nc))
        const = ctx.enter_context(tc.tile_pool(name="const", bufs=1))
        wpool = ctx.enter_context(tc.tile_pool(name="wpool", bufs=8))
        hpool = ctx.enter_context(tc.tile_pool(name="hpool", bufs=1))
        spool = ctx.enter_context(tc.tile_pool(name="spool", bufs=2))
        tiny = ctx.enter_context(tc.tile_pool(name="tiny", bufs=1))
        psA = ctx.enter_context(tc.tile_pool(name="psA", bufs=2, space="PSUM"))
        psB = ctx.enter_context(tc.tile_pool(name="psB", bufs=3, space="PSUM"))

        # input DMAs for all clusters up front (processing order first) so a
        # cluster's pt/xt land before the previous cluster's weight stream
        # monopolizes the DMA lanes
        pts, xts, wgts = {}, {}, {}
        for i in ORDER:
            pt = const.tile([128, KD * PD[i]], BF16, name=f"pt{i}")
            nc.gpsimd.dma_start(pt, pT_d[i][:, :, :])
            xt = const.tile([128, KD * CAP[i]], BF16, name=f"xt{i}")
            nc.gpsimd.dma_start(xt, xT_d[:, :, SLOT0[i]:SLOT0[i] + CAP[i]])
            pts[i], xts[i] = pt, xt
        for i in ORDER:
            for ti in range(TILES[i]):
                wgt = const.tile([128, PD[i] + 1], BF16, name=f"wgt{i}_{ti}")
                nc.gpsimd.dma_start(wgt, wg_d[i][ti, :, :])
                wgts[(i, ti)] = wgt

        # Stage A of cluster i+1 is emitted interleaved between cluster i's
        # stage-B pairs (one A psum-group per pair), so the PE does next-
        # cluster A work while ScalarE keeps streaming the current cluster's
        # exps instead of idling at the cluster boundary. Only pure compute
        # (matmul + cast/copy) is interleaved; DMAs stay in the prologue and
        # the DVE dot chain runs at the head of the cluster's own B section.
        state = {}  # i -> (hidT4, hts)

        def stage_A_steps(i):
            ntok, t0, kp, pd, nti = CAP[i], SLOT0[i], KP[i], PD[i], TILES[i]
            ng = kp // 2
            pt3 = pts[i].rearrange("p (k m) -> p k m", k=KD)
            xt3 = xts[i].rearrange("p (k t) -> p k t", k=KD)

            # ---- Stage A1: hiddenT [pd, ntok] as fp8 (x4), pd-major ----
            hidT = hpool.tile([128, kp * ntok], FP8, name=f"hidT{i}")
            hidT3 = hidT.rearrange("p (k t) -> p k t", k=kp)
            hidT4 = hidT.rearrange("p (g j t) -> p g j t", g=ng, j=2)
            hts = []
            state[i] = (hidT4, hts)
            for mp in range(kp):
                ps = psA.tile([128, ntok], F32, name=f"psA1_{i}_{mp}", tag="psA")
                for k in range(KD):
                    nc.tensor.matmul(
                        ps,
                        lhsT=pt3[:, k, mp * 128:(mp + 1) * 128],
                        rhs=xt3[:, k, :],
                        start=(k == 0),
                        stop=(k == KD - 1),
                    )
                # f32 -> fp8 with x4 pre-scale, on ScalarE
                nc.scalar.mul(hidT3[:, mp, :], ps, HSCALE)
                yield

            # ---- Stage A2: token-major hidden (bf16) ----
            for ti in range(nti):
                ht = hpool.tile([128, pd + 1], BF16, name=f"ht{i}_{ti}")
                hts.append(ht)
                nc.vector.memset(ht[:, pd:pd + 1], 1.0)
                for c0 in range(0, pd, 512):
                    cw = min(512, pd - c0)
                    ps = psA.tile([128, cw], F32, name=f"psA2_{i}_{ti}_{c0}", tag="psA")
                    for k in range(KD):
                        nc.tensor.matmul(
                            ps,
                            lhsT=xt3[:, k, ti * 128:(ti + 1) * 128],
                            rhs=pt3[:, k, c0:c0 + cw],
                            start=(k == 0),
                            stop=(k == KD - 1),
                        )
                    nc.vector.tensor_copy(ht[:, c0:c0 + cw], ps)
                    yield

        def stage_B(i, next_A):
            ntok, t0, kp, pd, nti = CAP[i], SLOT0[i], KP[i], PD[i], TILES[i]
            ng = kp // 2
            hidT4, hts = state[i]

            # target-logit dot: nlt = -(hidden . w[tgt] + b[tgt])
            nlts = []
            for ti in range(nti):
                wgt = wgts[(i, ti)]
                # absorb the DMA wait on a TensorCopy so the TensorTensor
                # below needs at most one sem wait
                dm = tiny.tile([128, 1], BF16, name=f"dm{i}_{ti}")
                nc.vector.tensor_copy(dm, wgt[:, 0:1])
                prod = spool.tile([128, pd + 1], F32, name=f"prod{i}_{ti}", tag="prod")
                nlt = tiny.tile([128, 1], F32, name=f"nlt{i}_{ti}")
                nc.vector.tensor_mul(prod, hts[ti], wgt)
                nc.vector.reduce_sum(nlt, prod, axis=X, negate=True)
                nlts.append(nlt)

            # ---- Stage B: fp8 DoubleRow logits + exp + running bf16 sum ----
            # chunks processed in pairs sharing one 2-bank PSUM tile; the exp
            # runs once per 1024 columns, and the exp'd pair is ADDED into a
            # running [128, 1024] bf16 accumulator (tensor_add at 2x bf16
            # rate); one small reduce per token tile happens in the finals
            npair = len(CHUNKS[i]) // 2
            accs = [
                hpool.tile([128, 2 * CHUNK], BF16, name=f"acc{i}_{ti}")
                for ti in range(nti)
            ]
            for cpair in range(npair):
                wt = wpool.tile([128, kp * CHUNK * 2], FP8, name=f"wt{i}_{cpair}",
                                tag="wt")
                off = kp * CHUNK * 2 * cpair
                nc.gpsimd.dma_start(wt, wT_d[i][:, off:off + kp * CHUNK * 2])
                wt4 = wt.rearrange("p (h g j c) -> p h g j c", h=2, g=ng, j=2)
                for ti in range(nti):
                    ps = psB.tile([128, 2 * CHUNK], F32,
                                  name=f"psB_{i}_{cpair}_{ti}", tag="psB")
                    for h in range(2):
                        for g in range(ng):
                            nc.tensor.matmul(
                                ps[:, h * CHUNK:(h + 1) * CHUNK],
                                lhsT=hidT4[:, g, :, ti * 128:(ti + 1) * 128],
                                rhs=wt4[:, h, g, :, :],
                                start=(g == 0),
                                stop=(g == ng - 1),
                                perf_mode=DR,
                            )
                    scr = spool.tile([128, 2 * CHUNK], BF16,
                                     name=f"scr{i}_{cpair}_{ti}", tag="scr", bufs=4)
                    nc.scalar.activation(scr, ps, Exp, scale=EXP_SCALE)
                    if cpair == 0:
                        nc.vector.tensor_copy(accs[ti], scr)
                    else:
                        nc.vector.tensor_add(accs[ti], accs[ti], scr)
                if next_A is not None and cpair >= 1:
                    next(next_A, None)

            # drain any remaining next-cluster A steps
            if next_A is not None:
                for _ in next_A:
                    pass

            # ---- Finals: nll = ln(sum exp - n_pad) - logit_t - bias ----
            npad_t = tiny.tile([128, 1], F32, name=f"npad{i}")
            nc.vector.memset(npad_t, float(-NPAD[i]))
            for ti in range(nti):
                S = tiny.tile([128, 1], F32, name=f"S{i}_{ti}")
                nc.vector.reduce_sum(S, accs[ti], axis=X)
                lse = tiny.tile([128, 1], F32, name=f"lse{i}_{ti}")
                nc.scalar.activation(lse, S, Ln, bias=npad_t)
                nllt = tiny.tile([128, 1], F32, name=f"nllt{i}_{ti}")
                nc.scalar.add(nllt, lse, nlts[ti])
                gt = t0 // 128 + ti
                nc.gpsimd.dma_start(out_d[gt:gt + 1, :], nllt)

        # first cluster's A runs as prologue; each B interleaves the next A
        for _ in stage_A_steps(ORDER[0]):
            pass
        for idx, i in enumerate(ORDER):
            nxt = stage_A_steps(ORDER[idx + 1]) if idx + 1 < len(ORDER) else None
            stage_B(i, nxt)

    nc.finalize()
    return nc


def _get_graph():
    if "nc" not in _GRAPH_CACHE:
        _GRAPH_CACHE["nc"] = _build_graph()
    return _GRAPH_CACHE["nc"]


def _pack_shared(ps, ws):
    """Core-independent packed params (broadcast to every core)."""
    pT_host, wT_host = [], []
    for i in range(3):
        pt = ps[i].T.reshape(KD, 128, PD[i]).transpose(1, 0, 2)  # [128, KD, pd]
        pT_host.append(np.ascontiguousarray(pt).astype(bf16))
        wk = ws[i].T.reshape(KP[i], 128, CSIZE[i]) * np.float32(WSCALE)
        wk = np.concatenate(
            [wk, np.zeros((KP[i], 128, NPAD[i]), np.float32)], axis=2
        )  # pad classes to CPAD with zero weights
        blocks = [
            wk[:, :, off:off + cs].transpose(1, 0, 2).reshape(128, -1)
            for (off, cs) in CHUNKS[i]
        ]
        wT_host.append(np.ascontiguousarray(np.concatenate(blocks, axis=1)).astype(fp8))
    return pT_host, wT_host


def kernel(x, target, p0, w0, b0, p1, w1, b1, p2, w2, b2):
    x = np.asarray(x, dtype=np.float32)
    tgt = np.asarray(target).astype(np.int64)
    ps = [np.asarray(p, np.float32) for p in (p0, p1, p2)]
    ws = [np.asarray(w, np.float32) for w in (w0, w1, w2)]
    bs = [np.asarray(b, np.float32) for b in (b0, b1, b2)]
    N = x.shape[0]

    cid = (tgt >= CUT[1]).astype(np.int32) + (tgt >= CUT[2]).astype(np.int32)
    perm = np.argsort(cid, kind="stable")
    segs = [perm[cid[perm] == i] for i in range(3)]
    core_idx = [np.array_split(segs[i], NCORES) for i in range(3)]
    for i in range(3):
        for j in range(NCORES):
            if len(core_idx[i][j]) > CAP[i]:
                raise RuntimeError(
                    f"cluster {i} capacity exceeded on core {j}: "
                    f"{len(core_idx[i][j])} > {CAP[i]}"
                )

    pT_host, wT_host = _pack_shared(ps, ws)

    in_maps = []
    for j in range(NCORES):
        Xp = np.zeros((TOT, D), np.float32)
        m = {}
        for i in range(3):
            tk = core_idx[i][j]
            l = len(tk)
            Xp[SLOT0[i]:SLOT0[i] + l] = x[tk]
            wg = np.zeros((TILES[i] * 128, PD[i] + 1), np.float32)
            if l:
                local_t = (tgt[tk] - CUT[i]).astype(np.int64)
                wg[:l, :PD[i]] = ws[i][local_t]
                wg[:l, PD[i]] = bs[i][local_t]
            m[f"wg{i}"] = np.ascontiguousarray(
                wg.reshape(TILES[i], 128, PD[i] + 1)
            ).astype(bf16)
            m[f"pT{i}"] = pT_host[i]
            m[f"wT{i}"] = wT_host[i]
        xt = Xp.T.reshape(KD, 128, TOT).transpose(1, 0, 2)  # [128, KD, TOT]
        m["xT"] = np.ascontiguousarray(xt).astype(bf16)
        in_maps.append(m)

    nc = _get_graph()
    res = run_bass_kernel_spmd(nc, in_maps, core_ids=list(range(NCORES)))
    _GRAPH_CACHE["last_results"] = res  # for external profiling harnesses

    nll = np.zeros((N,), np.float32)
    for j in range(NCORES):
        flat = np.asarray(res.results[j]["out"], np.float32).reshape(TOT)
        for i in range(3):
            tk = core_idx[i][j]
            nll[tk] = flat[SLOT0[i]:SLOT0[i] + len(tk)]
    loss = np.float32(nll.sum(dtype=np.float32))
    return loss, nll


# revision 48
# speedup vs baseline: 1.2053x; 1.2053x over previous
# Adaptive softmax (3-cluster) on 8 TRN2 NeuronCores.
#
# Strategy (moe_routing): each token only needs its own cluster's pipeline.
# Host-side we sort tokens by cluster (pure data movement), shard each
# cluster's token segment evenly over the 8 cores, and pad each per-core
# segment to a static capacity so the Bass graph stays shape-static.
# Per core, per cluster c with nt tokens / pd proj dim / C classes:
#   hiddenT[pd, nt] = p_c @ x_shard^T            (PE, bf16)
#   hidden  [nt, pd]                             (PE, bf16; token-major)
#   logits  [nt, C] = hidden @ w_c^T             (PE, fp8 DoubleRow, chunked)
#   sumexp  [nt]    = sum_C exp(logits)          (ScalarE exp -> DVE row-sum)
#   logit_t [nt]    = rowdot(hidden, w_c[tgt]) + b_c[tgt]    (DVE, bf16)
#   nll     [nt]    = ln(sumexp) - logit_t
# The big logits GEMM runs in fp8e4m3 DoubleRow (2 rows/PE cell = 2x MACs).
# fp8 range handling: weights are pre-scaled x64 and hidden x4 (powers of
# two, lossless), and the exp activation rescales with scale=1/256.
# No max-subtraction is needed: |logits| <~ 4 for this problem's scales.
# Classes are padded to uniform 512-wide chunks with zero weights; each
# padded class contributes exactly exp(0)=1, corrected via the Ln pre-bias.
# The target-row weights w_c[tgt] are gathered on host (data movement only).
# No collectives: pure data parallelism; host gathers/unpermutes/sums.

import numpy as np
import ml_dtypes
from contextlib import ExitStack

import concourse.bass as bass
import concourse.bacc as bacc
import concourse.mybir as mybir
import concourse.tile as tile
from concourse.bass_utils import run_bass_kernel_spmd

BF16 = mybir.dt.bfloat16
FP8 = mybir.dt.float8e4
F32 = mybir.dt.float32
bf16 = ml_dtypes.bfloat16
fp8 = ml_dtypes.float8_e4m3

VOCAB = 50257
D = 1024           # input dim
KD = D // 128      # k-tiles over input dim
CUT = [0, 10000, 30000, VOCAB]
PD = [1024, 512, 256]            # per-cluster projection dims
KP = [p // 128 for p in PD]      # k-tiles over proj dim
CSIZE = [CUT[i + 1] - CUT[i] for i in range(3)]
NCORES = 8
CAP = [256, 512, 512]            # per-core token capacity per cluster (padded)
TILES = [c // 128 for c in CAP]
SLOT0 = [0, 256, 768]            # slot offset of each cluster's segment
TOT = sum(CAP)                   # 1280 padded tokens per core
NT = TOT // 128                  # 10 token tiles per core
CHUNK = 512                      # class chunk (one PSUM bank of f32)

HSCALE = 4.0                     # hidden fp8 pre-scale (power of 2)
WSCALE = 64.0                    # weight fp8 pre-scale (power of 2)
EXP_SCALE = 1.0 / (HSCALE * WSCALE)

# class padding to uniform chunks
CPAD = [-(-C // CHUNK) * CHUNK for C in CSIZE]
NPAD = [CPAD[i] - CSIZE[i] for i in range(3)]
CHUNKS = [[(off, CHUNK) for off in range(0, CPAD[i], CHUNK)] for i in range(3)]

# cluster processing order: smallest input first (shrinks the pre-matmul
# head), c0 last (smallest tail)
ORDER = [2, 1, 0]

_GRAPH_CACHE = {}


def _build_graph():
    # Bacc (not plain Bass): its compile() pass splits semaphore waits into
    # event-semaphore carriers, satisfying TRN2's 1-wait-per-instruction limit.
    nc = bacc.Bacc(trn_type="TRN2", target_bir_lowering=False)

    xT_d = nc.dram_tensor("xT", [128, KD, TOT], BF16, kind="ExternalInput")
    pT_d = [
        nc.dram_tensor(f"pT{i}", [128, KD, PD[i]], BF16, kind="ExternalInput")
        for i in range(3)
    ]
    F = [KP[i] * CPAD[i] for i in range(3)]
    wT_d = [
        nc.dram_tensor(f"wT{i}", [128, F[i]], FP8, kind="ExternalInput")
        for i in range(3)
    ]
    # wg carries the gathered target-row weights plus one bias column
    wg_d = [
        nc.dram_tensor(f"wg{i}", [TILES[i], 128, PD[i] + 1], BF16, kind="ExternalInput")
        for i in range(3)
    ]
    out_d = nc.dram_tensor("out", [NT, 128], F32, kind="ExternalOutput")

    Exp = mybir.ActivationFunctionType.Exp
    Ln = mybir.ActivationFunctionType.Ln
    X = mybir.AxisListType.X
    DR = mybir.MatmulPerfMode.DoubleRow

    with ExitStack() as ctx:
        tc = ctx.enter_context(tile.TileContext(nc))
        const = ctx.enter_context(tc.tile_pool(name="const", bufs=1))
        wpool = ctx.enter_context(tc.tile_pool(name="wpool", bufs=8))
        hpool = ctx.enter_context(tc.tile_pool(name="hpool", bufs=1))
        spool = ctx.enter_context(tc.tile_pool(name="spool", bufs=2))
        tiny = ctx.enter_context(tc.tile_pool(name="tiny", bufs=1))
        psA = ctx.enter_context(tc.tile_pool(name="psA", bufs=2, space="PSUM"))
        psB = ctx.enter_context(tc.tile_pool(name="psB", bufs=3, space="PSUM"))

        # input DMAs for all clusters up front (processing order first) so a
        # cluster's pt/xt land before the previous cluster's weight stream
        # monopolizes the DMA lanes
        pts, xts = {}, {}
        for i in ORDER:
            pt = const.tile([128, KD * PD[i]], BF16, name=f"pt{i}")
            nc.gpsimd.dma_start(pt, pT_d[i][:, :, :])
            xt = const.tile([128, KD * CAP[i]], BF16, name=f"xt{i}")
            nc.gpsimd.dma_start(xt, xT_d[:, :, SLOT0[i]:SLOT0[i] + CAP[i]])
            pts[i], xts[i] = pt, xt

        for i in ORDER:
            ntok, t0, kp, pd, nti = CAP[i], SLOT0[i], KP[i], PD[i], TILES[i]
            ng = kp // 2  # DoubleRow contraction groups (K=256 each)

            pt3 = pts[i].rearrange("p (k m) -> p k m", k=KD)
            xt3 = xts[i].rearrange("p (k t) -> p k t", k=KD)

            # ---- Stage A1: hiddenT [pd, ntok] as fp8 (x4), pd-major ----
            hidT = hpool.tile([128, kp * ntok], FP8, name=f"hidT{i}")
            hidT3 = hidT.rearrange("p (k t) -> p k t", k=kp)
            hidT4 = hidT.rearrange("p (g j t) -> p g j t", g=ng, j=2)
            for mp in range(kp):
                ps = psA.tile([128, ntok], F32, name=f"psA1_{i}_{mp}", tag="psA")
                for k in range(KD):
                    nc.tensor.matmul(
                        ps,
                        lhsT=pt3[:, k, mp * 128:(mp + 1) * 128],
                        rhs=xt3[:, k, :],
                        start=(k == 0),
                        stop=(k == KD - 1),
                    )
                # f32 -> fp8 with x4 pre-scale, on ScalarE
                nc.scalar.mul(hidT3[:, mp, :], ps, HSCALE)

            # ---- Stage A2: token-major hidden (bf16) + target-logit dot ----
            nlts = []
            for ti in range(nti):
                ht = hpool.tile([128, pd + 1], BF16, name=f"ht{i}_{ti}")
                nc.vector.memset(ht[:, pd:pd + 1], 1.0)
                for c0 in range(0, pd, 512):
                    cw = min(512, pd - c0)
                    ps = psA.tile([128, cw], F32, name=f"psA2_{i}_{ti}_{c0}", tag="psA")
                    for k in range(KD):
                        nc.tensor.matmul(
                            ps,
                            lhsT=xt3[:, k, ti * 128:(ti + 1) * 128],
                            rhs=pt3[:, k, c0:c0 + cw],
                            start=(k == 0),
                            stop=(k == KD - 1),
                        )
                    nc.vector.tensor_copy(ht[:, c0:c0 + cw], ps)
                wgt = const.tile([128, pd + 1], BF16, name=f"wgt{i}_{ti}")
                nc.gpsimd.dma_start(wgt, wg_d[i][ti, :, :])
                # absorb the DMA wait on a TensorCopy so the TensorTensor
                # below needs at most one sem wait
                dm = tiny.tile([128, 1], BF16, name=f"dm{i}_{ti}")
                nc.vector.tensor_copy(dm, wgt[:, 0:1])
                prod = spool.tile([128, pd + 1], F32, name=f"prod{i}_{ti}", tag="prod")
                nlt = tiny.tile([128, 1], F32, name=f"nlt{i}_{ti}")
                nc.vector.tensor_mul(prod, ht, wgt)
                nc.vector.reduce_sum(nlt, prod, axis=X, negate=True)
                nlts.append(nlt)

            # ---- Stage B: fp8 DoubleRow logits + exp + running bf16 sum ----
            # chunks processed in pairs sharing one 2-bank PSUM tile; the exp
            # runs once per 1024 columns, and the exp'd pair is ADDED into a
            # running [128, 1024] bf16 accumulator (tensor_add at 2x bf16
            # rate); one small reduce per token tile happens in the finals
            npair = len(CHUNKS[i]) // 2
            accs = [
                hpool.tile([128, 2 * CHUNK], BF16, name=f"acc{i}_{ti}")
                for ti in range(nti)
            ]
            for cpair in range(npair):
                wt = wpool.tile([128, kp * CHUNK * 2], FP8, name=f"wt{i}_{cpair}",
                                tag="wt")
                off = kp * CHUNK * 2 * cpair
                nc.gpsimd.dma_start(wt, wT_d[i][:, off:off + kp * CHUNK * 2])
                wt4 = wt.rearrange("p (h g j c) -> p h g j c", h=2, g=ng, j=2)
                for ti in range(nti):
                    ps = psB.tile([128, 2 * CHUNK], F32,
                                  name=f"psB_{i}_{cpair}_{ti}", tag="psB")
                    for h in range(2):
                        for g in range(ng):
                            nc.tensor.matmul(
                                ps[:, h * CHUNK:(h + 1) * CHUNK],
                                lhsT=hidT4[:, g, :, ti * 128:(ti + 1) * 128],
                                rhs=wt4[:, h, g, :, :],
                                start=(g == 0),
                                stop=(g == ng - 1),
                                perf_mode=DR,
                            )
                    scr = spool.tile([128, 2 * CHUNK], BF16,
                                     name=f"scr{i}_{cpair}_{ti}", tag="scr", bufs=3)
                    nc.scalar.activation(scr, ps, Exp, scale=EXP_SCALE)
                    if cpair == 0:
                        nc.vector.tensor_copy(accs[ti], scr)
                    else:
                        nc.vector.tensor_add(accs[ti], accs[ti], scr)

            # ---- Finals: nll = ln(sum exp - n_pad) - logit_t - bias ----
            npad_t = tiny.tile([128, 1], F32, name=f"npad{i}")
            nc.vector.memset(npad_t, float(-NPAD[i]))
            for ti in range(nti):
                S = tiny.tile([128, 1], F32, name=f"S{i}_{ti}")
                nc.vector.reduce_sum(S, accs[ti], axis=X)
                lse = tiny.tile([128, 1], F32, name=f"lse{i}_{ti}")
                nc.scalar.activation(lse, S, Ln, bias=npad_t)
                nllt = tiny.tile([128, 1], F32, name=f"nllt{i}_{ti}")
                nc.scalar.add(nllt, lse, nlts[ti])
                gt = t0 // 128 + ti
                nc.gpsimd.dma_start(out_d[gt:gt + 1, :], nllt)

    nc.finalize()
    return nc


def _get_graph():
    if "nc" not in _GRAPH_CACHE:
        _GRAPH_CACHE["nc"] = _build_graph()
    return _GRAPH_CACHE["nc"]


def _pack_shared(ps, ws):
    """Core-independent packed params (broadcast to every core)."""
    pT_host, wT_host = [], []
    for i in range(3):
        pt = ps[i].T.reshape(KD, 128, PD[i]).transpose(1, 0, 2)  # [128, KD, pd]
        pT_host.append(np.ascontiguousarray(pt).astype(bf16))
        wk = ws[i].T.reshape(KP[i], 128, CSIZE[i]) * np.float32(WSCALE)
        wk = np.concatenate(
            [wk, np.zeros((KP[i], 128, NPAD[i]), np.float32)], axis=2
        )  # pad classes to CPAD with zero weights
        blocks = [
            wk[:, :, off:off + cs].transpose(1, 0, 2).reshape(128, -1)
            for (off, cs) in CHUNKS[i]
        ]
        wT_host.append(np.ascontiguousarray(np.concatenate(blocks, axis=1)).astype(fp8))
    return pT_host, wT_host


def kernel(x, target, p0, w0, b0, p1, w1, b1, p2, w2, b2):
    x = np.asarray(x, dtype=np.float32)
    tgt = np.asarray(target).astype(np.int64)
    ps = [np.asarray(p, np.float32) for p in (p0, p1, p2)]
    ws = [np.asarray(w, np.float32) for w in (w0, w1, w2)]
    bs = [np.asarray(b, np.float32) for b in (b0, b1, b2)]
    N = x.shape[0]

    cid = (tgt >= CUT[1]).astype(np.int32) + (tgt >= CUT[2]).astype(np.int32)
    perm = np.argsort(cid, kind="stable")
    segs = [perm[cid[perm] == i] for i in range(3)]
    core_idx = [np.array_split(segs[i], NCORES) for i in range(3)]
    for i in range(3):
        for j in range(NCORES):
            if len(core_idx[i][j]) > CAP[i]:
                raise RuntimeError(
                    f"cluster {i} capacity exceeded on core {j}: "
                    f"{len(core_idx[i][j])} > {CAP[i]}"
                )

    pT_host, wT_host = _pack_shared(ps, ws)

    in_maps = []
    for j in range(NCORES):
        Xp = np.zeros((TOT, D), np.float32)
        m = {}
        for i in range(3):
            tk = core_idx[i][j]
            l = len(tk)
            Xp[SLOT0[i]:SLOT0[i] + l] = x[tk]
            wg = np.zeros((TILES[i] * 128, PD[i] + 1), np.float32)
            if l:
                local_t = (tgt[tk] - CUT[i]).astype(np.int64)
                wg[:l, :PD[i]] = ws[i][local_t]
                wg[:l, PD[i]] = bs[i][local_t]
            m[f"wg{i}"] = np.ascontiguousarray(
                wg.reshape(TILES[i], 128, PD[i] + 1)
            ).astype(bf16)
            m[f"pT{i}"] = pT_host[i]
            m[f"wT{i}"] = wT_host[i]
        xt = Xp.T.reshape(KD, 128, TOT).transpose(1, 0, 2)  # [128, KD, TOT]
        m["xT"] = np.ascontiguousarray(xt).astype(bf16)
        in_maps.append(m)

    nc = _get_graph()
    res = run_bass_kernel_spmd(nc, in_maps, core_ids=list(range(NCORES)))
    _GRAPH_CACHE["last_results"] = res  # for external profiling harnesses

    nll = np.zeros((N,), np.float32)
    for j in range(NCORES):
        flat = np.asarray(res.results[j]["out"], np.float32).reshape(TOT)
        for i in range(3):
            tk = core_idx[i][j]
            nll[tk] = flat[SLOT0[i]:SLOT0[i] + len(tk)]
    loss = np.float32(nll.sum(dtype=np.float32))
    return loss, nll
